# revision 29
# baseline (speedup 1.0000x reference)
"""minGRU Trainium2 Bass kernel (fp8 DoubleRow + two-level residual quant).

Reference computation (per batch b):
    hidden = x @ W_hidden            [S, Di]
    gate   = x @ W_gate              [S, Di]
    a_t    = sigmoid(-gate)          (= 1 - z)
    z_t    = sigmoid(gate)
    g(h)   = h + 0.5 if h >= 0 else sigmoid(h)
    b_t    = z_t * g(hidden_t)
    h_t    = a_t * h_{t-1} + b_t     (h_{-1} = 0; linear-space scan)
    out    = h @ W_out               [S, D]

Sharding over 8 cores: (batch b in 0..3) x (half of Di). Each core computes
its batch's projections against its 768-column slice of W_hidden/W_gate,
scans, and multiplies by its 768-row slice of W_out, producing a partial
[D, S] (transposed) output. Host adds the two halves and transposes back.

Precision strategy (error budget: scale-rel max err < 2e-2):
  - gate proj: all 8 k-tiles fp8e4m3 DoubleRow pairs (sigmoid + scan damp
    the noise; cheapest error per saved cycle).
  - hidden proj: k-tiles 0..2 in fp8 "residual" DR mode — the two DR slots
    hold (W8, V8) with V8 = fp8(W*SW - W8) and the rhs broadcasts the same
    x8 tile into both slots (stride-0 dim1), so PSUM gets x8 @ (W8+V8):
    two-level weight quantization that cancels the weight-side error.
    k-tiles 3..7 in fp16 (same PE speed as bf16, 8x the mantissa).
  - out proj: all 6 f-tiles in residual DR mode on mean-centered
    dh8 = fp8(16*(h - c)), c = per-feature mean of h over chunk 0
    (computed on device; correction c@W_out folded into the PSUM->SBUF
    copy as a per-partition bias).
  - elementwise z/s/a/g/b in fp16 (unlocks DVE 2x/4x modes).
  Measured stream cost: DR-pair MMs and fp16 MMs interleave so the DR
  weight loads hide under the fp16 streams (~14.1us/chunk PE).
  Scales: fp8 weights x32, dh x16 -> out-proj PSUM holds 512*out; the
  copy applies (po + 512*c@Wo) * (1/512).
"""

import numpy as np
import ml_dtypes
from contextlib import ExitStack

import concourse.bass as bass
import concourse.tile as tile
from concourse import bacc, mybir
from concourse.bass_utils import run_bass_kernel_spmd

B = 4
S = 4096
D = 1024
DI = 1536
F = DI // 2            # 768 features per core
N_CORES = 8
SC = 512               # sequence chunk (one PSUM bank of fp32)
KD = D // 128          # 8 contraction tiles for the projections
NF = F // 128          # 6 feature tiles per core
ND = D // 128          # 8 output d-model tiles

ABLATE = 0             # 1: drains only (no scan/ew); 2: also no out-proj
HR = 3                 # hidden k-tiles in residual-DR mode (rest fp16)
NCOPY_DVE = 0          # out-proj PSUM->SBUF copies routed to DVE (rest Act)
GP_A = True            # a = 1-z on gpsimd (else DVE)
GP_B = True            # b = z*g on gpsimd (else DVE)
GP_G = False           # g = max(u, s) on gpsimd (else DVE)
GP_D8 = False          # d8 quantize on gpsimd (else DVE)
U_ACT = True           # u = h + 0.5 on Act (frees ph via one engine)

SW = 32.0              # fp8 weight scale
SH = 16.0              # dh scale
OS = 512.0             # out-proj PSUM scale (= SW * SH)

F32 = mybir.dt.float32
BF16 = mybir.dt.bfloat16
FP16 = mybir.dt.float16
FP8 = mybir.dt.float8e4
ACT = mybir.ActivationFunctionType
ALU = mybir.AluOpType
DR = mybir.MatmulPerfMode.DoubleRow

NP_FP8 = ml_dtypes.float8_e4m3
NP_BF16 = ml_dtypes.bfloat16

_cache = {}


def _emit_out_dt(nc, d8_sb, sc, dt_, wor_sb, vd_sb, opool, spool, outT):
    """Residual-DR out-projection + biased PSUM->SBUF copy + store for one
    128-row d-model tile of one chunk."""
    po = opool.tile([128, SC], F32, tag="po")
    for fk in range(NF):
        dbc = d8_sb[:, fk, :].unsqueeze(1).broadcast_to([128, 2, SC])
        nc.tensor.matmul(
            po[:], wor_sb[:, 2 * fk:2 * fk + 2, dt_ * 128:(dt_ + 1) * 128],
            dbc, perf_mode=DR, start=(fk == 0), stop=(fk == NF - 1))
    o_sb = spool.tile([128, SC], FP16, tag="o")
    # out = (po + 512*c@Wo) / 512, fp16 (host adds the halves in fp32)
    if dt_ < ND - NCOPY_DVE:
        nc.scalar.activation(
            o_sb[:], po[:], ACT.Identity,
            bias=vd_sb[:, dt_:dt_ + 1], scale=1.0 / OS)
    else:
        nc.vector.tensor_scalar(
            o_sb[:], po[:], 1.0 / OS, vd_sb[:, dt_:dt_ + 1],
            ALU.mult, ALU.add)
    r0 = (sc * ND + dt_) * 128
    nc.sync.dma_start(outT[r0:r0 + 128, :], o_sb[:])


def _build(seq_len=S, reps=1, timing=False):
    nsc = seq_len // SC
    nc = bacc.Bacc("TRN2", target_bir_lowering=False, debug=False,
                   num_devices=N_CORES)
    kind_in = None if timing else "ExternalInput"
    kind_out = None if timing else "ExternalOutput"

    def dram(name, shape, dt, kind):
        if kind is None:
            return nc.dram_tensor(name, shape, dt).ap()
        return nc.dram_tensor(name, shape, dt, kind=kind).ap()

    nsc_io = seq_len // SC
    # chunk-major layouts: each (chunk, tile) block is 128 contiguous rows
    # of SC so the DMA coalesces instead of 128 strided 512B descriptors.
    x8 = dram("x8", [nsc_io * KD * 128, SC], FP8, kind_in)
    xh = dram("xh", [nsc_io * (KD - HR) * 128, SC], FP16, kind_in)
    wg8 = dram("wg8", [KD * 128, F], FP8, kind_in)
    whr = dram("whr", [2 * HR * 128, F], FP8, kind_in)
    whh = dram("whh", [(KD - HR) * 128, F], FP16, kind_in)
    wor = dram("wor", [2 * NF * 128, D], FP8, kind_in)
    wob = dram("wob", [F, D], BF16, kind_in)
    outT = dram("outT", [nsc_io * ND * 128, SC], FP16, kind_out)
    if timing:
        seed = nc.dram_tensor("seed", [1, 8], F32, kind="ExternalInput").ap()
        done = nc.dram_tensor("done", [1, 8 * reps], F32,
                              kind="ExternalOutput").ap()

    with tile.TileContext(nc) as tc, ExitStack() as ctx:
        wpool = ctx.enter_context(tc.tile_pool(name="w", bufs=1))
        xpool = ctx.enter_context(tc.tile_pool(name="x", bufs=3))
        ppool = ctx.enter_context(tc.tile_pool(name="pp", bufs=3, space="PSUM"))
        opool = ctx.enter_context(tc.tile_pool(name="po", bufs=2, space="PSUM"))
        vpool = opool
        epool = ctx.enter_context(tc.tile_pool(name="e", bufs=3))
        hpool = ctx.enter_context(tc.tile_pool(name="h", bufs=2))
        dpool = ctx.enter_context(tc.tile_pool(name="d8", bufs=2))
        spool = ctx.enter_context(tc.tile_pool(name="os", bufs=3))
        cpool = ctx.enter_context(tc.tile_pool(name="c", bufs=1))

        # Resident weights; dim1 indexes the 128-row k-tile (or (W8,V8)
        # residual slot pairs for the DR modes).
        wg8_sb = wpool.tile([128, KD, F], FP8, tag="wg8")
        for dk in range(KD):
            nc.sync.dma_start(wg8_sb[:, dk, :], wg8[dk * 128:(dk + 1) * 128, :])
        whr_sb = wpool.tile([128, 2 * HR, F], FP8, tag="whr")
        for dk in range(2 * HR):
            nc.sync.dma_start(whr_sb[:, dk, :], whr[dk * 128:(dk + 1) * 128, :])
        whh_sb = wpool.tile([128, KD - HR, F], FP16, tag="whh")
        for dk in range(KD - HR):
            nc.sync.dma_start(whh_sb[:, dk, :], whh[dk * 128:(dk + 1) * 128, :])
        wor_sb = wpool.tile([128, 2 * NF, D], FP8, tag="wor")
        for fk in range(2 * NF):
            nc.sync.dma_start(wor_sb[:, fk, :], wor[fk * 128:(fk + 1) * 128, :])
        wob_sb = wpool.tile([128, NF, D], BF16, tag="wob")
        for fk in range(NF):
            nc.sync.dma_start(wob_sb[:, fk, :], wob[fk * 128:(fk + 1) * 128, :])

        # Per-partition bias tiles for the centered out-proj (chunk-0 c).
        negc_sb = cpool.tile([128, NF], F32, tag="negc")    # -16*c per f-tile
        v_sb = cpool.tile([128, ND], F32, tag="v")          # 512*c@Wo
        vd_sb = cpool.tile([128, ND], F32, tag="vd")        # c@Wo
        half_sb = cpool.tile([128, 1], F32, tag="half")
        nc.vector.memset(half_sb[:], 0.5)

        for _rep in range(reps):
          h_prev = [None] * NF
          prev = None      # (d8_sb, sc) awaiting out-proj
          x_tiles = {}

          def _load_x(sc_):
              x8_sb = xpool.tile([128, KD, SC], FP8, tag="x8")
              for dk in range(KD):
                  r0 = (sc_ * KD + dk) * 128
                  nc.sync.dma_start(x8_sb[:, dk, :], x8[r0:r0 + 128, :])
              xh_sb = xpool.tile([128, KD - HR, SC], FP16, tag="xh")
              for dk in range(KD - HR):
                  r0 = (sc_ * (KD - HR) + dk) * 128
                  nc.sync.dma_start(xh_sb[:, dk, :], xh[r0:r0 + 128, :])
              x_tiles[sc_] = (x8_sb, xh_sb)

          _load_x(0)
          if nsc > 1:
              _load_x(1)
          for sc in range(nsc):
            if sc + 2 < nsc:
                _load_x(sc + 2)
            x8_sb, xh_sb = x_tiles.pop(sc)

            # out-proj of the previous chunk, interleaved between ft-groups
            out_sched = {1: [0], 2: [1, 2], 3: [3], 4: [4, 5], 5: [6, 7]}

            # bias matmul must precede the first interleaved out-proj copy
            if sc == 1 and not ABLATE:
                pv = vpool.tile([128, SC], F32, tag="po")
                for dt_ in range(ND):
                    for fk in range(NF):
                        nc.tensor.matmul(
                            pv[:, dt_:dt_ + 1],
                            wob_sb[:, fk, dt_ * 128:(dt_ + 1) * 128],
                            cb_sb[:, fk:fk + 1],
                            start=(fk == 0), stop=(fk == NF - 1))
                nc.vector.tensor_copy(v_sb[:], pv[:, :ND])
                nc.scalar.activation(vd_sb[:], v_sb[:], ACT.Copy,
                                     scale=1.0 / OS)

            h_cur = []
            ab_q = []
            d8_sb = dpool.tile([128, NF, SC], FP8, tag="d8")

            def _emit_scan(ft_, sc=sc, h_prev=h_prev, h_cur=h_cur, ab_q=ab_q):
                a_t, b_t = ab_q[ft_]
                h_sb = hpool.tile([128, SC], F32, tag=f"h{ft_}")
                init = 0.0 if sc == 0 else h_prev[ft_][:, SC - 1:SC]
                nc.vector.tensor_tensor_scan(
                    h_sb[:], a_t[:], b_t[:], init,
                    op0=ALU.mult, op1=ALU.add)
                h_cur.append(h_sb)

            if ABLATE:
                nc.vector.memset(d8_sb[:], 0.25)
            for ft in range(NF):
                # interleave the previous chunk's out-proj dt-groups between
                # this chunk's ft-groups to smooth PSUM/Act/DMA pressure
                if prev is not None and ABLATE < 2:
                    for dt_ in out_sched.get(ft, []):
                        _emit_out_dt(nc, prev[0], prev[1], dt_, wor_sb,
                                     vd_sb, opool, spool, outT)
                ph = ppool.tile([128, SC], F32, tag="ph")
                pg = ppool.tile([128, SC], F32, tag="pg")
                cw = ft * 128
                # Interleave DR-heavy MMs (gate pairs + hidden residual)
                # with fp16 MMs so DR weight loads hide under fp16 streams.
                dr_part = [("g", dk) for dk in range(0, KD, 2)]
                dr_part += [("r", i) for i in range(HR)]
                h_part = [("h", i) for i in range(KD - HR)]
                seq = []
                while dr_part or h_part:
                    if dr_part:
                        seq.append(dr_part.pop(0))
                    if h_part:
                        seq.append(h_part.pop(0))
                gi = hi = 0
                ng, nh = KD // 2, HR + (KD - HR)
                for kind, idx in seq:
                    if kind == "g":
                        nc.tensor.matmul(
                            pg[:], wg8_sb[:, idx:idx + 2, cw:cw + 128],
                            x8_sb[:, idx:idx + 2, :], perf_mode=DR,
                            start=(gi == 0), stop=(gi == ng - 1))
                        gi += 1
                    elif kind == "r":
                        xbc = x8_sb[:, idx, :].unsqueeze(1).broadcast_to(
                            [128, 2, SC])
                        nc.tensor.matmul(
                            ph[:], whr_sb[:, 2 * idx:2 * idx + 2, cw:cw + 128],
                            xbc, perf_mode=DR,
                            start=(hi == 0), stop=(hi == nh - 1))
                        hi += 1
                    else:
                        nc.tensor.matmul(
                            ph[:], whh_sb[:, idx, cw:cw + 128],
                            xh_sb[:, idx, :],
                            start=(hi == 0), stop=(hi == nh - 1))
                        hi += 1

                if ABLATE:
                    zz = epool.tile([128, SC], FP16, tag="zz")
                    nc.scalar.activation(zz[:], pg[:], ACT.Sigmoid,
                                         scale=1.0 / SW)
                    ss = epool.tile([128, SC], FP16, tag="ss")
                    nc.scalar.activation(ss[:], ph[:], ACT.Sigmoid,
                                         scale=1.0 / SW)
                    continue
                z_sb = epool.tile([128, SC], FP16, tag="z")
                s_sb = epool.tile([128, SC], FP16, tag="s")
                u_sb = epool.tile([128, SC], FP16, tag="u")
                a_sb = epool.tile([128, SC], FP16, tag="a")
                g_sb = epool.tile([128, SC], FP16, tag="g")
                b_sb = epool.tile([128, SC], FP16, tag="b")
                # Projections are scaled by SW in PSUM; descale at the
                # PSUM readers (Act sigmoids, DVE u). PSUM readers must be
                # Act or DVE (gpsimd has no PSUM access).
                nc.scalar.activation(z_sb[:], pg[:], ACT.Sigmoid,
                                     scale=1.0 / SW)
                nc.scalar.activation(s_sb[:], ph[:], ACT.Sigmoid,
                                     scale=1.0 / SW)
                # u = h + 0.5
                if U_ACT:
                    nc.scalar.activation(u_sb[:], ph[:], ACT.Identity,
                                         bias=half_sb[:], scale=1.0 / SW)
                else:
                    nc.vector.tensor_scalar(u_sb[:], ph[:], 1.0 / SW, 0.5,
                                            ALU.mult, ALU.add)
                # a = 1 - z (fp16 SBUF-only -> fast mode)
                eng_a = nc.gpsimd if GP_A else nc.vector
                eng_a.tensor_scalar(a_sb[:], z_sb[:], -1.0, 1.0,
                                    ALU.mult, ALU.add)
                # g = max(h + 0.5, sigmoid(h))
                eng_g = nc.gpsimd if GP_G else nc.vector
                eng_g.tensor_tensor(g_sb[:], u_sb[:], s_sb[:], op=ALU.max)
                # b = z * g
                eng_b = nc.gpsimd if GP_B else nc.vector
                eng_b.tensor_mul(b_sb[:], z_sb[:], g_sb[:])

                ab_q.append((a_sb, b_sb))
                # Emit each scan one ft late so the DVE stream never stalls
                # waiting on gpsimd's b of the same ft.
                if ft > 0:
                    _emit_scan(ft - 1)
                continue
            if not ABLATE:
                _emit_scan(NF - 1)

            if sc == 0 and not ABLATE:
                # c = per-feature mean of h over chunk 0 (bias matmuls are
                # deferred to the next chunk to keep the PE stream busy).
                cb_sb = cpool.tile([128, NF], BF16, tag="cb")
                for fk in range(NF):
                    hsum = epool.tile([128, 1], F32, tag="hsum")
                    nc.vector.tensor_reduce(
                        hsum[:], h_cur[fk][:], mybir.AxisListType.X, ALU.add)
                    nc.vector.tensor_scalar(negc_sb[:, fk:fk + 1], hsum[:],
                                            -SH / SC, None, ALU.mult)
                    nc.vector.tensor_scalar(cb_sb[:, fk:fk + 1], hsum[:],
                                            1.0 / SC, None, ALU.mult)

            eng_d = nc.gpsimd if GP_D8 else nc.vector
            for ft in range(NF) if not ABLATE else []:
                # dh8 = fp8(16*h - 16*c) (SBUF-only -> 2x mode)
                eng_d.tensor_scalar(
                    d8_sb[:, ft, :], h_cur[ft][:],
                    SH, negc_sb[:, ft:ft + 1], ALU.mult, ALU.add)

            if nsc == 1:
                # single-chunk build: bias + out-proj inline
                pv = vpool.tile([128, SC], F32, tag="po")
                for dt_ in range(ND):
                    for fk in range(NF):
                        nc.tensor.matmul(
                            pv[:, dt_:dt_ + 1],
                            wob_sb[:, fk, dt_ * 128:(dt_ + 1) * 128],
                            cb_sb[:, fk:fk + 1],
                            start=(fk == 0), stop=(fk == NF - 1))
                nc.vector.tensor_copy(v_sb[:], pv[:, :ND])
                nc.scalar.activation(vd_sb[:], v_sb[:], ACT.Copy,
                                     scale=1.0 / OS)
                for dt_ in range(ND):
                    _emit_out_dt(nc, d8_sb, sc, dt_, wor_sb, vd_sb,
                                 opool, spool, outT)
            else:
                prev = (d8_sb, sc)
            h_prev = h_cur

          if prev is not None:
            (p_d8, p_sc) = prev
            for dt_ in range(ND):
                _emit_out_dt(nc, p_d8, p_sc, dt_, wor_sb, vd_sb,
                             opool, spool, outT)

          if timing and _rep == reps - 1:
            tok = spool.tile([1, 8 * reps], F32, tag="tok")
            nc.vector.memset(tok[:], 1.0)
            nc.sync.dma_start(done[:], tok[:])

    nc.compile()
    return nc


def get_nc(seq_len=S, reps=1, timing=False):
    key = (seq_len, reps, timing)
    if key not in _cache:
        _cache[key] = _build(seq_len, reps, timing)
    return _cache[key]


def _q8(a):
    return np.clip(a, -240, 240).astype(NP_FP8)


def make_in_maps(x, W_hidden, W_gate, W_out):
    """Shard full inputs into per-core input maps (core c -> batch c//2,
    Di-half c%2)."""
    in_maps = []
    xT = np.ascontiguousarray(np.transpose(x, (0, 2, 1)))        # [B, D, S]
    # chunk-major repack: [D, S] -> [nsc*KD*128, SC]
    nsc = S // SC

    def chunk_major(a):                      # [rows, S] -> [nsc*rows, SC]
        rows = a.shape[0]
        return np.ascontiguousarray(
            a.reshape(rows, nsc, SC).transpose(1, 0, 2).reshape(-1, SC))

    xT8 = np.stack([chunk_major(xT[b].astype(NP_FP8)) for b in range(B)])
    xTh = np.stack([chunk_major(xT[b, HR * 128:].astype(np.float16))
                    for b in range(B)])
    for c in range(N_CORES):
        b, hf = divmod(c, 2)
        Wg = W_gate[:, hf * F:(hf + 1) * F]
        Wh = W_hidden[:, hf * F:(hf + 1) * F]
        Wo = W_out[hf * F:(hf + 1) * F, :]
        m = {}
        m["x8"] = xT8[b]
        m["xh"] = np.ascontiguousarray(xTh[b])
        m["wg8"] = _q8(Wg * SW)
        # hidden residual tiles: interleave (W8, V8) per k-tile
        whr = np.empty((2 * HR * 128, F), NP_FP8)
        for i in range(HR):
            Wt = Wh[i * 128:(i + 1) * 128] * SW
            W8 = _q8(Wt)
            whr[2 * i * 128:(2 * i + 1) * 128] = W8
            whr[(2 * i + 1) * 128:(2 * i + 2) * 128] = _q8(
                Wt - W8.astype(np.float32))
        m["whr"] = whr
        m["whh"] = np.ascontiguousarray(Wh[HR * 128:] * SW).astype(np.float16)
        # out residual tiles: interleave (W8, V8) per f-tile
        wor = np.empty((2 * NF * 128, D), NP_FP8)
        for i in range(NF):
            Wt = Wo[i * 128:(i + 1) * 128] * SW
            W8 = _q8(Wt)
            wor[2 * i * 128:(2 * i + 1) * 128] = W8
            wor[(2 * i + 1) * 128:(2 * i + 2) * 128] = _q8(
                Wt - W8.astype(np.float32))
        m["wor"] = wor
        m["wob"] = np.ascontiguousarray(Wo * OS).astype(NP_BF16)
        in_maps.append(m)
    return in_maps


def assemble(results):
    """Combine per-core partial transposed outputs into [B, S, D]."""
    nsc = S // SC
    out = np.empty((B, S, D), np.float32)
    for b in range(B):
        acc = (results[2 * b]["outT"].astype(np.float32)
               + results[2 * b + 1]["outT"].astype(np.float32))
        # [nsc*ND*128, SC] chunk-major -> [D, S]
        acc = acc.reshape(nsc, D, SC).transpose(1, 0, 2).reshape(D, S)
        out[b] = acc.T
    return out


def kernel(x, W_hidden, W_gate, W_out):
    x = np.asarray(x, np.float32)
    W_hidden = np.asarray(W_hidden, np.float32)
    W_gate = np.asarray(W_gate, np.float32)
    W_out = np.asarray(W_out, np.float32)
    nc = get_nc()
    in_maps = make_in_maps(x, W_hidden, W_gate, W_out)
    last_err = None
    for attempt in range(3):
        try:
            res = run_bass_kernel_spmd(nc, in_maps, list(range(N_CORES)))
            return assemble(res.results)
        except Exception as e:  # transient device faults under axon
            last_err = e
            import time as _time
            _time.sleep(5.0 * (attempt + 1))
    raise last_err


# revision 31
# speedup vs baseline: 1.2282x; 1.2282x over previous
"""minGRU Trainium2 Bass kernel (fp8 DoubleRow + two-level residual quant).

Reference computation (per batch b):
    hidden = x @ W_hidden            [S, Di]
    gate   = x @ W_gate              [S, Di]
    a_t    = sigmoid(-gate)          (= 1 - z)
    z_t    = sigmoid(gate)
    g(h)   = h + 0.5 if h >= 0 else sigmoid(h)
    b_t    = z_t * g(hidden_t)
    h_t    = a_t * h_{t-1} + b_t     (h_{-1} = 0; linear-space scan)
    out    = h @ W_out               [S, D]

Sharding over 8 cores: (batch b in 0..3) x (half of Di). Each core computes
its batch's projections against its 768-column slice of W_hidden/W_gate,
scans, and multiplies by its 768-row slice of W_out, producing a partial
[D, S] (transposed) output. Host adds the two halves and transposes back.

Precision strategy (error budget: scale-rel max err < 2e-2):
  - gate proj: all 8 k-tiles fp8e4m3 DoubleRow pairs (sigmoid + scan damp
    the noise; cheapest error per saved cycle).
  - hidden proj: k-tiles 0..2 in fp8 "residual" DR mode — the two DR slots
    hold (W8, V8) with V8 = fp8(W*SW - W8) and the rhs broadcasts the same
    x8 tile into both slots (stride-0 dim1), so PSUM gets x8 @ (W8+V8):
    two-level weight quantization that cancels the weight-side error.
    k-tiles 3..7 in fp16 (same PE speed as bf16, 8x the mantissa).
  - out proj: all 6 f-tiles in residual DR mode on mean-centered
    dh8 = fp8(16*(h - c)), c = per-feature mean of h over chunk 0
    (computed on device; correction c@W_out folded into the PSUM->SBUF
    copy as a per-partition bias).
  - elementwise z/s/a/g/b in fp16 (unlocks DVE 2x/4x modes).
  Measured stream cost: DR-pair MMs and fp16 MMs interleave so the DR
  weight loads hide under the fp16 streams (~14.1us/chunk PE).
  Scales: fp8 weights x32, dh x16 -> out-proj PSUM holds 512*out; the
  copy applies (po + 512*c@Wo) * (1/512).
"""

import numpy as np
import ml_dtypes
from contextlib import ExitStack

import concourse.bass as bass
import concourse.tile as tile
from concourse import bacc, mybir
from concourse.bass_utils import run_bass_kernel_spmd

B = 4
S = 4096
D = 1024
DI = 1536
F = DI // 2            # 768 features per core
N_CORES = 8
SC = 512               # sequence chunk (one PSUM bank of fp32)
KD = D // 128          # 8 contraction tiles for the projections
NF = F // 128          # 6 feature tiles per core
ND = D // 128          # 8 output d-model tiles

ABLATE = 0             # 1: drains only (no scan/ew); 2: also no out-proj
HR = 3                 # hidden k-tiles in residual-DR mode (rest fp16)
NCOPY_DVE = 0          # out-proj PSUM->SBUF copies routed to DVE (rest Act)
GP_A = True            # a = 1-z on gpsimd (else DVE)
GP_B = True            # b = z*g on gpsimd (else DVE)
GP_G = False           # g = max(u, s) on gpsimd (else DVE)
GP_D8 = False          # d8 quantize on gpsimd (else DVE)
U_ACT = True           # u = h + 0.5 on Act (frees ph via one engine)
INTERLEAVE_OUT = False # out-proj dt-groups interleaved between ft-groups
PP_BUFS = 2            # pg/ph PSUM buffer generations
OP_BUFS = 3            # po PSUM buffer generations

SW = 32.0              # fp8 weight scale
SH = 16.0              # dh scale
OS = 512.0             # out-proj PSUM scale (= SW * SH)

F32 = mybir.dt.float32
BF16 = mybir.dt.bfloat16
FP16 = mybir.dt.float16
FP8 = mybir.dt.float8e4
ACT = mybir.ActivationFunctionType
ALU = mybir.AluOpType
DR = mybir.MatmulPerfMode.DoubleRow

NP_FP8 = ml_dtypes.float8_e4m3
NP_BF16 = ml_dtypes.bfloat16

_cache = {}


def _emit_out_dt(nc, d8_sb, sc, dt_, wor_sb, vd_sb, opool, spool, outT):
    """Residual-DR out-projection + biased PSUM->SBUF copy + store for one
    128-row d-model tile of one chunk."""
    po = opool.tile([128, SC], F32, tag="po")
    for fk in range(NF):
        dbc = d8_sb[:, fk, :].unsqueeze(1).broadcast_to([128, 2, SC])
        nc.tensor.matmul(
            po[:], wor_sb[:, 2 * fk:2 * fk + 2, dt_ * 128:(dt_ + 1) * 128],
            dbc, perf_mode=DR, start=(fk == 0), stop=(fk == NF - 1))
    o_sb = spool.tile([128, SC], FP16, tag="o")
    # out = (po + 512*c@Wo) / 512, fp16 (host adds the halves in fp32)
    if dt_ < ND - NCOPY_DVE:
        nc.scalar.activation(
            o_sb[:], po[:], ACT.Identity,
            bias=vd_sb[:, dt_:dt_ + 1], scale=1.0 / OS)
    else:
        nc.vector.tensor_scalar(
            o_sb[:], po[:], 1.0 / OS, vd_sb[:, dt_:dt_ + 1],
            ALU.mult, ALU.add)
    r0 = (sc * ND + dt_) * 128
    nc.sync.dma_start(outT[r0:r0 + 128, :], o_sb[:])


def _build(seq_len=S, reps=1, timing=False):
    nsc = seq_len // SC
    nc = bacc.Bacc("TRN2", target_bir_lowering=False, debug=False,
                   num_devices=N_CORES)
    kind_in = None if timing else "ExternalInput"
    kind_out = None if timing else "ExternalOutput"

    def dram(name, shape, dt, kind):
        if kind is None:
            return nc.dram_tensor(name, shape, dt).ap()
        return nc.dram_tensor(name, shape, dt, kind=kind).ap()

    nsc_io = seq_len // SC
    # chunk-major layouts: each (chunk, tile) block is 128 contiguous rows
    # of SC so the DMA coalesces instead of 128 strided 512B descriptors.
    x8 = dram("x8", [nsc_io * KD * 128, SC], FP8, kind_in)
    xh = dram("xh", [nsc_io * (KD - HR) * 128, SC], FP16, kind_in)
    wg8 = dram("wg8", [KD * 128, F], FP8, kind_in)
    whr = dram("whr", [2 * HR * 128, F], FP8, kind_in)
    whh = dram("whh", [(KD - HR) * 128, F], FP16, kind_in)
    wor = dram("wor", [2 * NF * 128, D], FP8, kind_in)
    wob = dram("wob", [F, D], BF16, kind_in)
    outT = dram("outT", [nsc_io * ND * 128, SC], FP16, kind_out)
    if timing:
        seed = nc.dram_tensor("seed", [1, 8], F32, kind="ExternalInput").ap()
        done = nc.dram_tensor("done", [1, 8 * reps], F32,
                              kind="ExternalOutput").ap()

    with tile.TileContext(nc) as tc, ExitStack() as ctx:
        wpool = ctx.enter_context(tc.tile_pool(name="w", bufs=1))
        xpool = ctx.enter_context(tc.tile_pool(name="x", bufs=3))
        ppool = ctx.enter_context(tc.tile_pool(name="pp", bufs=PP_BUFS,
                                                space="PSUM"))
        opool = ctx.enter_context(tc.tile_pool(name="po", bufs=OP_BUFS,
                                               space="PSUM"))
        vpool = opool
        epool = ctx.enter_context(tc.tile_pool(name="e", bufs=3))
        hpool = ctx.enter_context(tc.tile_pool(name="h", bufs=2))
        dpool = ctx.enter_context(tc.tile_pool(name="d8", bufs=2))
        spool = ctx.enter_context(tc.tile_pool(name="os", bufs=3))
        cpool = ctx.enter_context(tc.tile_pool(name="c", bufs=1))

        # Resident weights; dim1 indexes the 128-row k-tile (or (W8,V8)
        # residual slot pairs for the DR modes).
        wg8_sb = wpool.tile([128, KD, F], FP8, tag="wg8")
        for dk in range(KD):
            nc.sync.dma_start(wg8_sb[:, dk, :], wg8[dk * 128:(dk + 1) * 128, :])
        whr_sb = wpool.tile([128, 2 * HR, F], FP8, tag="whr")
        for dk in range(2 * HR):
            nc.sync.dma_start(whr_sb[:, dk, :], whr[dk * 128:(dk + 1) * 128, :])
        whh_sb = wpool.tile([128, KD - HR, F], FP16, tag="whh")
        for dk in range(KD - HR):
            nc.sync.dma_start(whh_sb[:, dk, :], whh[dk * 128:(dk + 1) * 128, :])
        wor_sb = wpool.tile([128, 2 * NF, D], FP8, tag="wor")
        for fk in range(2 * NF):
            nc.sync.dma_start(wor_sb[:, fk, :], wor[fk * 128:(fk + 1) * 128, :])
        wob_sb = wpool.tile([128, NF, D], BF16, tag="wob")
        for fk in range(NF):
            nc.sync.dma_start(wob_sb[:, fk, :], wob[fk * 128:(fk + 1) * 128, :])

        # Per-partition bias tiles for the centered out-proj (chunk-0 c).
        negc_sb = cpool.tile([128, NF], F32, tag="negc")    # -16*c per f-tile
        v_sb = cpool.tile([128, ND], F32, tag="v")          # 512*c@Wo
        vd_sb = cpool.tile([128, ND], F32, tag="vd")        # c@Wo
        half_sb = cpool.tile([128, 1], F32, tag="half")
        nc.vector.memset(half_sb[:], 0.5)

        for _rep in range(reps):
          h_prev = [None] * NF
          prev = None      # (d8_sb, sc) awaiting out-proj
          x_tiles = {}

          def _load_x(sc_):
              x8_sb = xpool.tile([128, KD, SC], FP8, tag="x8")
              for dk in range(KD):
                  r0 = (sc_ * KD + dk) * 128
                  nc.sync.dma_start(x8_sb[:, dk, :], x8[r0:r0 + 128, :])
              xh_sb = xpool.tile([128, KD - HR, SC], FP16, tag="xh")
              for dk in range(KD - HR):
                  r0 = (sc_ * (KD - HR) + dk) * 128
                  nc.sync.dma_start(xh_sb[:, dk, :], xh[r0:r0 + 128, :])
              x_tiles[sc_] = (x8_sb, xh_sb)

          _load_x(0)
          if nsc > 1:
              _load_x(1)
          for sc in range(nsc):
            if sc + 2 < nsc:
                _load_x(sc + 2)
            x8_sb, xh_sb = x_tiles.pop(sc)

            # out-proj of the previous chunk, interleaved between ft-groups
            out_sched = {1: [0], 2: [1, 2], 3: [3], 4: [4, 5], 5: [6, 7]}

            # bias matmul must precede the first interleaved out-proj copy
            if sc == 1 and not ABLATE:
                pv = vpool.tile([128, SC], F32, tag="po")
                for dt_ in range(ND):
                    for fk in range(NF):
                        nc.tensor.matmul(
                            pv[:, dt_:dt_ + 1],
                            wob_sb[:, fk, dt_ * 128:(dt_ + 1) * 128],
                            cb_sb[:, fk:fk + 1],
                            start=(fk == 0), stop=(fk == NF - 1))
                nc.vector.tensor_copy(v_sb[:], pv[:, :ND])
                nc.scalar.activation(vd_sb[:], v_sb[:], ACT.Copy,
                                     scale=1.0 / OS)

            h_cur = []
            ab_q = []
            d8_sb = dpool.tile([128, NF, SC], FP8, tag="d8")

            def _emit_scan(ft_, sc=sc, h_prev=h_prev, h_cur=h_cur, ab_q=ab_q):
                a_t, b_t = ab_q[ft_]
                h_sb = hpool.tile([128, SC], F32, tag=f"h{ft_}")
                init = 0.0 if sc == 0 else h_prev[ft_][:, SC - 1:SC]
                nc.vector.tensor_tensor_scan(
                    h_sb[:], a_t[:], b_t[:], init,
                    op0=ALU.mult, op1=ALU.add)
                h_cur.append(h_sb)

            if ABLATE:
                nc.vector.memset(d8_sb[:], 0.25)
            for ft in range(NF):
                # interleave the previous chunk's out-proj dt-groups between
                # this chunk's ft-groups to smooth PSUM/Act/DMA pressure
                if prev is not None and ABLATE < 2 and INTERLEAVE_OUT:
                    for dt_ in out_sched.get(ft, []):
                        _emit_out_dt(nc, prev[0], prev[1], dt_, wor_sb,
                                     vd_sb, opool, spool, outT)
                ph = ppool.tile([128, SC], F32, tag="ph")
                pg = ppool.tile([128, SC], F32, tag="pg")
                cw = ft * 128
                # Interleave DR-heavy MMs (gate pairs + hidden residual)
                # with fp16 MMs so DR weight loads hide under fp16 streams.
                dr_part = [("g", dk) for dk in range(0, KD, 2)]
                dr_part += [("r", i) for i in range(HR)]
                h_part = [("h", i) for i in range(KD - HR)]
                seq = []
                while dr_part or h_part:
                    if dr_part:
                        seq.append(dr_part.pop(0))
                    if h_part:
                        seq.append(h_part.pop(0))
                gi = hi = 0
                ng, nh = KD // 2, HR + (KD - HR)
                for kind, idx in seq:
                    if kind == "g":
                        nc.tensor.matmul(
                            pg[:], wg8_sb[:, idx:idx + 2, cw:cw + 128],
                            x8_sb[:, idx:idx + 2, :], perf_mode=DR,
                            start=(gi == 0), stop=(gi == ng - 1))
                        gi += 1
                    elif kind == "r":
                        xbc = x8_sb[:, idx, :].unsqueeze(1).broadcast_to(
                            [128, 2, SC])
                        nc.tensor.matmul(
                            ph[:], whr_sb[:, 2 * idx:2 * idx + 2, cw:cw + 128],
                            xbc, perf_mode=DR,
                            start=(hi == 0), stop=(hi == nh - 1))
                        hi += 1
                    else:
                        nc.tensor.matmul(
                            ph[:], whh_sb[:, idx, cw:cw + 128],
                            xh_sb[:, idx, :],
                            start=(hi == 0), stop=(hi == nh - 1))
                        hi += 1

                if ABLATE:
                    zz = epool.tile([128, SC], FP16, tag="zz")
                    nc.scalar.activation(zz[:], pg[:], ACT.Sigmoid,
                                         scale=1.0 / SW)
                    ss = epool.tile([128, SC], FP16, tag="ss")
                    nc.scalar.activation(ss[:], ph[:], ACT.Sigmoid,
                                         scale=1.0 / SW)
                    continue
                z_sb = epool.tile([128, SC], FP16, tag="z")
                s_sb = epool.tile([128, SC], FP16, tag="s")
                u_sb = epool.tile([128, SC], FP16, tag="u")
                a_sb = epool.tile([128, SC], FP16, tag="a")
                g_sb = epool.tile([128, SC], FP16, tag="g")
                b_sb = epool.tile([128, SC], FP16, tag="b")
                # Projections are scaled by SW in PSUM; descale at the
                # PSUM readers (Act sigmoids, DVE u). PSUM readers must be
                # Act or DVE (gpsimd has no PSUM access).
                nc.scalar.activation(z_sb[:], pg[:], ACT.Sigmoid,
                                     scale=1.0 / SW)
                nc.scalar.activation(s_sb[:], ph[:], ACT.Sigmoid,
                                     scale=1.0 / SW)
                # u = h + 0.5
                if U_ACT:
                    nc.scalar.activation(u_sb[:], ph[:], ACT.Identity,
                                         bias=half_sb[:], scale=1.0 / SW)
                else:
                    nc.vector.tensor_scalar(u_sb[:], ph[:], 1.0 / SW, 0.5,
                                            ALU.mult, ALU.add)
                # a = 1 - z (fp16 SBUF-only -> fast mode)
                eng_a = nc.gpsimd if GP_A else nc.vector
                eng_a.tensor_scalar(a_sb[:], z_sb[:], -1.0, 1.0,
                                    ALU.mult, ALU.add)
                # g = max(h + 0.5, sigmoid(h))
                eng_g = nc.gpsimd if GP_G else nc.vector
                eng_g.tensor_tensor(g_sb[:], u_sb[:], s_sb[:], op=ALU.max)
                # b = z * g
                eng_b = nc.gpsimd if GP_B else nc.vector
                eng_b.tensor_mul(b_sb[:], z_sb[:], g_sb[:])

                ab_q.append((a_sb, b_sb))
                # Emit each scan one ft late so the DVE stream never stalls
                # waiting on gpsimd's b of the same ft.
                if ft > 0:
                    _emit_scan(ft - 1)
                continue
            if not ABLATE:
                _emit_scan(NF - 1)

            if sc == 0 and not ABLATE:
                # c = per-feature mean of h over chunk 0 (bias matmuls are
                # deferred to the next chunk to keep the PE stream busy).
                cb_sb = cpool.tile([128, NF], BF16, tag="cb")
                for fk in range(NF):
                    hsum = epool.tile([128, 1], F32, tag="hsum")
                    nc.vector.tensor_reduce(
                        hsum[:], h_cur[fk][:], mybir.AxisListType.X, ALU.add)
                    nc.vector.tensor_scalar(negc_sb[:, fk:fk + 1], hsum[:],
                                            -SH / SC, None, ALU.mult)
                    nc.vector.tensor_scalar(cb_sb[:, fk:fk + 1], hsum[:],
                                            1.0 / SC, None, ALU.mult)

            eng_d = nc.gpsimd if GP_D8 else nc.vector
            for ft in range(NF) if not ABLATE else []:
                # dh8 = fp8(16*h - 16*c) (SBUF-only -> 2x mode)
                eng_d.tensor_scalar(
                    d8_sb[:, ft, :], h_cur[ft][:],
                    SH, negc_sb[:, ft:ft + 1], ALU.mult, ALU.add)

            if prev is not None and not INTERLEAVE_OUT:
                for dt_ in range(ND):
                    _emit_out_dt(nc, prev[0], prev[1], dt_, wor_sb,
                                 vd_sb, opool, spool, outT)
            if nsc == 1:
                # single-chunk build: bias + out-proj inline
                pv = vpool.tile([128, SC], F32, tag="po")
                for dt_ in range(ND):
                    for fk in range(NF):
                        nc.tensor.matmul(
                            pv[:, dt_:dt_ + 1],
                            wob_sb[:, fk, dt_ * 128:(dt_ + 1) * 128],
                            cb_sb[:, fk:fk + 1],
                            start=(fk == 0), stop=(fk == NF - 1))
                nc.vector.tensor_copy(v_sb[:], pv[:, :ND])
                nc.scalar.activation(vd_sb[:], v_sb[:], ACT.Copy,
                                     scale=1.0 / OS)
                for dt_ in range(ND):
                    _emit_out_dt(nc, d8_sb, sc, dt_, wor_sb, vd_sb,
                                 opool, spool, outT)
            else:
                prev = (d8_sb, sc)
            h_prev = h_cur

          if prev is not None:
            (p_d8, p_sc) = prev
            for dt_ in range(ND):
                _emit_out_dt(nc, p_d8, p_sc, dt_, wor_sb, vd_sb,
                             opool, spool, outT)

          if timing and _rep == reps - 1:
            tok = spool.tile([1, 8 * reps], F32, tag="tok")
            nc.vector.memset(tok[:], 1.0)
            nc.sync.dma_start(done[:], tok[:])

    nc.compile()
    return nc


def get_nc(seq_len=S, reps=1, timing=False):
    key = (seq_len, reps, timing)
    if key not in _cache:
        _cache[key] = _build(seq_len, reps, timing)
    return _cache[key]


def _q8(a):
    return np.clip(a, -240, 240).astype(NP_FP8)


def make_in_maps(x, W_hidden, W_gate, W_out):
    """Shard full inputs into per-core input maps (core c -> batch c//2,
    Di-half c%2)."""
    in_maps = []
    xT = np.ascontiguousarray(np.transpose(x, (0, 2, 1)))        # [B, D, S]
    # chunk-major repack: [D, S] -> [nsc*KD*128, SC]
    nsc = S // SC

    def chunk_major(a):                      # [rows, S] -> [nsc*rows, SC]
        rows = a.shape[0]
        return np.ascontiguousarray(
            a.reshape(rows, nsc, SC).transpose(1, 0, 2).reshape(-1, SC))

    xT8 = np.stack([chunk_major(xT[b].astype(NP_FP8)) for b in range(B)])
    xTh = np.stack([chunk_major(xT[b, HR * 128:].astype(np.float16))
                    for b in range(B)])
    for c in range(N_CORES):
        b, hf = divmod(c, 2)
        Wg = W_gate[:, hf * F:(hf + 1) * F]
        Wh = W_hidden[:, hf * F:(hf + 1) * F]
        Wo = W_out[hf * F:(hf + 1) * F, :]
        m = {}
        m["x8"] = xT8[b]
        m["xh"] = np.ascontiguousarray(xTh[b])
        m["wg8"] = _q8(Wg * SW)
        # hidden residual tiles: interleave (W8, V8) per k-tile
        whr = np.empty((2 * HR * 128, F), NP_FP8)
        for i in range(HR):
            Wt = Wh[i * 128:(i + 1) * 128] * SW
            W8 = _q8(Wt)
            whr[2 * i * 128:(2 * i + 1) * 128] = W8
            whr[(2 * i + 1) * 128:(2 * i + 2) * 128] = _q8(
                Wt - W8.astype(np.float32))
        m["whr"] = whr
        m["whh"] = np.ascontiguousarray(Wh[HR * 128:] * SW).astype(np.float16)
        # out residual tiles: interleave (W8, V8) per f-tile
        wor = np.empty((2 * NF * 128, D), NP_FP8)
        for i in range(NF):
            Wt = Wo[i * 128:(i + 1) * 128] * SW
            W8 = _q8(Wt)
            wor[2 * i * 128:(2 * i + 1) * 128] = W8
            wor[(2 * i + 1) * 128:(2 * i + 2) * 128] = _q8(
                Wt - W8.astype(np.float32))
        m["wor"] = wor
        m["wob"] = np.ascontiguousarray(Wo * OS).astype(NP_BF16)
        in_maps.append(m)
    return in_maps


def assemble(results):
    """Combine per-core partial transposed outputs into [B, S, D]."""
    nsc = S // SC
    out = np.empty((B, S, D), np.float32)
    for b in range(B):
        acc = (results[2 * b]["outT"].astype(np.float32)
               + results[2 * b + 1]["outT"].astype(np.float32))
        # [nsc*ND*128, SC] chunk-major -> [D, S]
        acc = acc.reshape(nsc, D, SC).transpose(1, 0, 2).reshape(D, S)
        out[b] = acc.T
    return out


def kernel(x, W_hidden, W_gate, W_out):
    x = np.asarray(x, np.float32)
    W_hidden = np.asarray(W_hidden, np.float32)
    W_gate = np.asarray(W_gate, np.float32)
    W_out = np.asarray(W_out, np.float32)
    nc = get_nc()
    in_maps = make_in_maps(x, W_hidden, W_gate, W_out)
    last_err = None
    for attempt in range(3):
        try:
            res = run_bass_kernel_spmd(nc, in_maps, list(range(N_CORES)))
            return assemble(res.results)
        except Exception as e:  # transient device faults under axon
            last_err = e
            import time as _time
            _time.sleep(5.0 * (attempt + 1))
    raise last_err
